# revision 12
# baseline (speedup 1.0000x reference)
"""Trainium2 Bass kernel for nn_MultiHeadAttention_79130477461654.

The reference einsum "nhqk,nhvd->nhqd" contracts k and v independently, so
out = (sum_k softmax(energy))*(sum_s v) = broadcast(sum_s v) since softmax
rows sum to 1.  With v = split_heads(x @ Wv) and the reference's direct
(n,h,q,d)->(n,s,e) reshape, the full output reduces to

    xs[n]    = sum_s x[n,s,:]                       (1024,)
    Sfull[n] = xs[n] @ Wv                           (1024,)
    WoSum    = sum_m Wo[64m+d, :]  (d=0..63)        (64, 1024)
    T[n,h,:] = Sfull[n][64h:64h+64] @ WoSum + bo    (16, 1024)
    out[n, 64h+r, :] = T[n,h,:]   for r in 0..63

which is numerically within ~1e-6 of the reference (softmax-row-sum
rounding).  Sharding: data parallel over batch N=8, one batch per core;
Wv/Wo replicated.  All arithmetic runs on-device.
"""

import numpy as np

N, S, E, H, D = 8, 1024, 1024, 16, 64
NCORES = 8
P = 128  # partitions


def build_nc():
    import concourse.bacc as bacc
    import concourse.mybir as mybir
    from concourse.tile import TileContext

    F32 = mybir.dt.float32
    nc = bacc.Bacc("TRN2", target_bir_lowering=False, debug=False)

    xd = nc.declare_dram_parameter("x", [S, E], F32, isOutput=False)
    wvd = nc.declare_dram_parameter("Wv", [E, E], F32, isOutput=False)
    wod = nc.declare_dram_parameter("Wo", [E, E], F32, isOutput=False)
    bod = nc.declare_dram_parameter("bo128", [P, E], F32, isOutput=False)
    onesd = nc.declare_dram_parameter("ones128", [P, 1], F32, isOutput=False)
    dbld = nc.declare_dram_parameter("dblI", [P, D], F32, isOutput=False)
    outd = nc.declare_dram_parameter("out", [S, E], F32, isOutput=True)

    with TileContext(nc) as tc:
        with (
            tc.tile_pool(name="xin", bufs=3) as xp,
            tc.tile_pool(name="wv", bufs=8) as wvp,
            tc.tile_pool(name="wo", bufs=8) as wop,
            tc.tile_pool(name="accs", bufs=1) as ap,
            tc.tile_pool(name="small", bufs=1) as sp,
            tc.tile_pool(name="psum", bufs=1, space="PSUM") as psp,
            tc.tile_pool(name="dram", bufs=1, space="DRAM") as dp,
        ):
            ones_sb = sp.tile([P, 1], F32)
            nc.sync.dma_start(out=ones_sb[:], in_=onesd[:])
            dbl_sb = sp.tile([P, D], F32)
            nc.sync.dma_start(out=dbl_sb[:], in_=dbld[:])
            bo_sb = sp.tile([P, E], F32)
            nc.sync.dma_start(out=bo_sb[:], in_=bod[:])

            # ---- xacc = sum over the 8 row-chunks of x  (128, 1024)
            xacc = ap.tile([P, E], F32)
            for i in range(8):
                xt = xp.tile([P, E], F32)
                nc.sync.dma_start(out=xt[:], in_=xd[i * P : (i + 1) * P, :])
                if i == 0:
                    nc.vector.tensor_copy(xacc[:], xt[:])
                else:
                    nc.vector.tensor_add(xacc[:], xacc[:], xt[:])

            # ---- Wv tiles (all live: consumed twice)
            wvt = []
            for c in range(8):
                t = wvp.tile([P, E], F32)
                nc.sync.dma_start(out=t[:], in_=wvd[c * P : (c + 1) * P, :])
                wvt.append(t)

            # ---- WoSum[d, :] = sum_m Wo[64m + d, :]  via PE with double identity
            wot = []
            for i in range(8):
                wt = wop.tile([P, E], F32)
                nc.sync.dma_start(out=wt[:], in_=wod[i * P : (i + 1) * P, :])
                wot.append(wt)
            ps_wosum = psp.tile([D, E], F32)
            for half in range(2):
                sl = slice(half * 512, half * 512 + 512)
                for i in range(8):
                    nc.tensor.matmul(
                        ps_wosum[:, sl],
                        dbl_sb[:],
                        wot[i][:, sl],
                        start=(i == 0),
                        stop=(i == 7),
                    )
            wosum = sp.tile([D, E], F32)
            nc.vector.tensor_copy(wosum[:], ps_wosum[:])

            # ---- xsT[p, c] = xs[128c + p]  via PE partition-reduction
            ps_xsT = psp.tile([P, 8], F32)
            for c in range(8):
                nc.tensor.matmul(
                    ps_xsT[:, c : c + 1],
                    xacc[:, c * P : (c + 1) * P],
                    ones_sb[:],
                    start=True,
                    stop=True,
                )
            xsT = sp.tile([P, 8], F32)
            nc.vector.tensor_copy(xsT[:], ps_xsT[:])

            # ---- Sfull row (1, 1024) = xs @ Wv
            ps_S = psp.tile([1, E], F32)
            for half in range(2):
                sl = slice(half * 512, half * 512 + 512)
                for c in range(8):
                    nc.tensor.matmul(
                        ps_S[0:1, sl],
                        xsT[:, c : c + 1],
                        wvt[c][:, sl],
                        start=(c == 0),
                        stop=(c == 7),
                    )
            srow = sp.tile([1, E], F32)
            nc.vector.tensor_copy(srow[:], ps_S[:])

            # ---- transpose on PE: SfT[d, h] = Sfull[64h + d]
            #      (K=1 matmul: out(64,1) = srow[0, 64h:64h+64].T @ ones(1,1))
            ps_sft = psp.tile([D, H], F32)
            for h in range(H):
                nc.tensor.matmul(
                    ps_sft[:, h : h + 1],
                    srow[0:1, h * D : (h + 1) * D],
                    ones_sb[0:1, 0:1],
                    start=True,
                    stop=True,
                )
            sft = sp.tile([D, H], F32)
            nc.vector.tensor_copy(sft[:], ps_sft[:])

            # ---- T (16, 1024) = SfT.T @ WoSum ; Tb = T + bo
            ps_T = psp.tile([H, E], F32)
            for half in range(2):
                sl = slice(half * 512, half * 512 + 512)
                nc.tensor.matmul(ps_T[:, sl], sft[:], wosum[:, sl], start=True, stop=True)
            tb = sp.tile([H, E], F32)
            nc.vector.tensor_add(tb[:], ps_T[:], bo_sb[0:H, :])

            # ---- broadcast rows: out[64h + r, :] = Tb[h, :]
            tbd = dp.tile([H, E], F32)
            nc.sync.dma_start(out=tbd[:], in_=tb[:])
            outr = outd.rearrange("(h r) e -> h r e", r=64)
            for g in range(8):
                nc.sync.dma_start(
                    out=outr[g * 2 : (g + 1) * 2],
                    in_=tbd[g * 2 : (g + 1) * 2, None, :].to_broadcast((2, 64, E)),
                )

    nc.compile()
    return nc


_NC_CACHE = None


def make_in_maps(x, Wv, Wo, bo):
    x = np.ascontiguousarray(np.asarray(x, dtype=np.float32))
    Wv = np.ascontiguousarray(np.asarray(Wv, dtype=np.float32))
    Wo = np.ascontiguousarray(np.asarray(Wo, dtype=np.float32))
    bo = np.ascontiguousarray(np.asarray(bo, dtype=np.float32))
    bo128 = np.tile(bo.reshape(1, E), (P, 1))
    ones128 = np.ones((P, 1), dtype=np.float32)
    dblI = np.zeros((P, D), dtype=np.float32)
    dblI[np.arange(P), np.arange(P) % D] = 1.0
    return [
        {
            "x": np.ascontiguousarray(x[j]),
            "Wv": Wv,
            "Wo": Wo,
            "bo128": bo128,
            "ones128": ones128,
            "dblI": dblI,
        }
        for j in range(NCORES)
    ]


def kernel(x, Wq=None, Wk=None, Wv=None, Wo=None, bo=None, **_unused):
    from concourse.bass_utils import run_bass_kernel_spmd

    global _NC_CACHE
    if _NC_CACHE is None:
        _NC_CACHE = build_nc()
    nc = _NC_CACHE

    in_maps = make_in_maps(x, Wv, Wo, bo)
    res = run_bass_kernel_spmd(nc, in_maps, core_ids=list(range(NCORES))).results
    return np.stack([res[j]["out"] for j in range(NCORES)], axis=0)


# revision 25
# speedup vs baseline: 1.9063x; 1.9063x over previous
"""Trainium2 Bass kernel for nn_MultiHeadAttention_79130477461654.

The reference einsum "nhqk,nhvd->nhqd" contracts k and v independently, so
out = (sum_k softmax(energy))*(sum_s v) = broadcast(sum_s v) since softmax
rows sum to 1.  With v = split_heads(x @ Wv) and the reference's direct
(n,h,q,d)->(n,s,e) reshape, the full output reduces to

    xs[n]    = sum_s x[n,s,:]                       (1024,)
    Sfull[n] = xs[n] @ Wv                           (1024,)
    WoSum    = sum_m Wo[64m+d, :]  (d=0..63)        (64, 1024)
    T[n,h,:] = Sfull[n][64h:64h+64] @ WoSum + bo    (16, 1024)
    out[n, 64h+r, :] = T[n,h,:]   for r in 0..63

which is numerically within ~1e-6 of the reference (softmax-row-sum
rounding).  Sharding: data parallel over batch N=8, one batch per core;
Wv/Wo replicated.  All arithmetic runs on-device.
"""

import numpy as np

N, S, E, H, D = 8, 1024, 1024, 16, 64
NCORES = 8
P = 128  # partitions


def build_nc():
    import concourse.bacc as bacc
    import concourse.mybir as mybir
    from concourse.tile import TileContext

    F32 = mybir.dt.float32
    nc = bacc.Bacc("TRN2", target_bir_lowering=False, debug=False)

    xd = nc.declare_dram_parameter("x", [S, E], F32, isOutput=False)
    wvd = nc.declare_dram_parameter("Wv", [E, E], F32, isOutput=False)
    wod = nc.declare_dram_parameter("Wo", [E, E], F32, isOutput=False)
    bod = nc.declare_dram_parameter("bo128", [P, E], F32, isOutput=False)
    onesd = nc.declare_dram_parameter("ones128", [P, 1], F32, isOutput=False)
    dbld = nc.declare_dram_parameter("dblI", [P, D], F32, isOutput=False)
    outd = nc.declare_dram_parameter("out", [S, E], F32, isOutput=True)

    F32R = mybir.dt.float32r

    with TileContext(nc) as tc:
        with (
            tc.tile_pool(name="xin", bufs=3) as xp,
            tc.tile_pool(name="wv", bufs=8) as wvp,
            tc.tile_pool(name="wo", bufs=3) as wop,
            tc.tile_pool(name="accs", bufs=1) as ap,
            tc.tile_pool(name="small", bufs=1) as sp,
            tc.tile_pool(name="outsb", bufs=3) as op,
            tc.tile_pool(name="psA", bufs=1, space="PSUM") as psA,
            tc.tile_pool(name="psB", bufs=1, space="PSUM") as psB,
            tc.tile_pool(name="psO", bufs=4, space="PSUM") as psO,
        ):
            ones_sb = sp.tile([P, 1], F32)
            nc.sync.dma_start(out=ones_sb[:], in_=onesd[:])
            dbl_sb = sp.tile([P, D], F32)
            nc.sync.dma_start(out=dbl_sb[:], in_=dbld[:])
            bo_sb = sp.tile([P, E], F32)
            nc.sync.dma_start(out=bo_sb[:], in_=bod[:])
            dbl_r = sp.tile([P, D], F32R)
            nc.vector.tensor_copy(dbl_r[:], dbl_sb[:])

            # ---- xacc = sum over the 8 row-chunks of x  (128, 1024)
            xacc = ap.tile([P, E], F32)
            for i in range(8):
                xt = xp.tile([P, E], F32)
                nc.sync.dma_start(out=xt[:], in_=xd[i * P : (i + 1) * P, :])
                if i == 0:
                    nc.vector.tensor_copy(xacc[:], xt[:])
                else:
                    nc.vector.tensor_add(xacc[:], xacc[:], xt[:])

            # ---- Wv tiles, rounded to fp32r for the wide Sfull matmuls
            wvr = []
            for c in range(8):
                t = wvp.tile([P, E], F32, tag="wvf")
                nc.sync.dma_start(out=t[:], in_=wvd[c * P : (c + 1) * P, :])
                tr = wvp.tile([P, E], F32R, tag="wvr")
                nc.vector.tensor_copy(tr[:], t[:])
                wvr.append(tr)

            # ---- woacc = sum over the 8 row-chunks of Wo (128, 1024)
            #      final add writes the fp32r-rounded copy for the PE fold
            woacc = ap.tile([P, E], F32)
            woacc_r = ap.tile([P, E], F32R)
            for i in range(8):
                wt = wop.tile([P, E], F32)
                nc.sync.dma_start(out=wt[:], in_=wod[i * P : (i + 1) * P, :])
                if i == 0:
                    nc.vector.tensor_copy(woacc[:], wt[:])
                elif i < 7:
                    nc.vector.tensor_add(woacc[:], woacc[:], wt[:])
                else:
                    nc.vector.tensor_add(woacc_r[:], woacc[:], wt[:])

            # ---- xsT[p, c] = xs[128c + p]  via PE partition-reduction (N=1, fp32)
            ps_xsT = psA.tile([P, 8], F32, tag="psa")
            for c in range(8):
                nc.tensor.matmul(
                    ps_xsT[:, c : c + 1],
                    xacc[:, c * P : (c + 1) * P],
                    ones_sb[:],
                    start=True,
                    stop=True,
                )
            xsT = sp.tile([P, 8], F32R)
            nc.vector.tensor_copy(xsT[:], ps_xsT[:])

            # ---- Sfull row (1, 1024) = xs @ Wv  (wide fp32r, chases Wv DMA)
            ps_S = psB.tile([1, E], F32, tag="psb")
            for c in range(8):
                for half in range(2):
                    sl = slice(half * 512, half * 512 + 512)
                    nc.tensor.matmul(
                        ps_S[0:1, sl],
                        xsT[:, c : c + 1],
                        wvr[c][:, sl],
                        start=(c == 0),
                        stop=(c == 7),
                        skip_group_check=True,
                    )
            srow = sp.tile([1, E], F32)
            nc.vector.tensor_copy(srow[:], ps_S[:])

            # ---- sft[d, h] = Sfull[64h + d]  (N=1 fp32, all at base partition 0)
            ps_sft = psA.tile([D, H], F32, tag="psa")
            for h in range(H):
                nc.tensor.matmul(
                    ps_sft[:, h : h + 1],
                    srow[0:1, h * D : (h + 1) * D],
                    ones_sb[0:1, 0:1],
                    start=True,
                    stop=True,
                )
            sft = sp.tile([D, H], F32)
            nc.vector.tensor_copy(sft[:], ps_sft[:])

            # ---- rep[d, 64h + r] = sft[d, h]  (DVE free-dim broadcast, fp32r out)
            rep = sp.tile([D, H * D], F32R)
            nc.vector.tensor_copy(
                rep[:].rearrange("d (h r) -> d h r", r=D),
                sft[:, :, None].to_broadcast((D, H, D)),
            )

            # ---- WoSum fold (64, 1024) = woacc[0:64] + woacc[64:128] via PE (fp32r)
            ps_fold = psB.tile([D, E], F32, tag="psb")
            for half in range(2):
                sl = slice(half * 512, half * 512 + 512)
                nc.tensor.matmul(
                    ps_fold[:, sl],
                    dbl_r[:],
                    woacc_r[:, sl],
                    start=True,
                    stop=True,
                )
            wosum = sp.tile([D, E], F32R)
            nc.vector.tensor_copy(wosum[:], ps_fold[:])

            # ---- fused T+broadcast: out_tile t (128 rows = heads 2t, 2t+1 each x64)
            #      out[p, :] = sum_d rep[d, 128t + p] * WoSum[d, :]   (fp32r)
            for t in range(8):
                ob = op.tile([P, E], F32)
                for half in range(2):
                    sl = slice(half * 512, half * 512 + 512)
                    po = psO.tile([P, 512], F32, tag="pso")
                    nc.tensor.matmul(
                        po[:],
                        rep[:, t * P : (t + 1) * P],
                        wosum[:, sl],
                        start=True,
                        stop=True,
                    )
                    # bias add fused with PSUM->SBUF move
                    nc.vector.tensor_add(ob[:, sl], po[:], bo_sb[:, sl])
                nc.sync.dma_start(out=outd[t * P : (t + 1) * P, :], in_=ob[:])

    nc.compile()
    return nc


_NC_CACHE = None


def make_in_maps(x, Wv, Wo, bo):
    x = np.ascontiguousarray(np.asarray(x, dtype=np.float32))
    Wv = np.ascontiguousarray(np.asarray(Wv, dtype=np.float32))
    Wo = np.ascontiguousarray(np.asarray(Wo, dtype=np.float32))
    bo = np.ascontiguousarray(np.asarray(bo, dtype=np.float32))
    bo128 = np.tile(bo.reshape(1, E), (P, 1))
    ones128 = np.ones((P, 1), dtype=np.float32)
    dblI = np.zeros((P, D), dtype=np.float32)
    dblI[np.arange(P), np.arange(P) % D] = 1.0
    return [
        {
            "x": np.ascontiguousarray(x[j]),
            "Wv": Wv,
            "Wo": Wo,
            "bo128": bo128,
            "ones128": ones128,
            "dblI": dblI,
        }
        for j in range(NCORES)
    ]


def kernel(x, Wq=None, Wk=None, Wv=None, Wo=None, bo=None, **_unused):
    from concourse.bass_utils import run_bass_kernel_spmd

    global _NC_CACHE
    if _NC_CACHE is None:
        _NC_CACHE = build_nc()
    nc = _NC_CACHE

    in_maps = make_in_maps(x, Wv, Wo, bo)
    res = run_bass_kernel_spmd(nc, in_maps, core_ids=list(range(NCORES))).results
    return np.stack([res[j]["out"] for j in range(NCORES)], axis=0)
